# revision 20
# baseline (speedup 1.0000x reference)
"""DetectionLoss Trainium2 Bass kernel (v3 - sparse-only, fused masks).

Data-parallel over batch: 2 images per core x 8 cores; host sums per-box
partials (npos is a global normalizer, so per-core normalization is
impossible anyway - the sharding hint's "per-shard sums + counts").

The CE term only needs logsumexp at the <=128 positive cells, so there is
no dense cls work at all: the host packs per-cell records
[obj, reg0..3, cls0..29] (pure relayout) and the device gathers 35-wide
rows at the box cells. lse is exp+accum+ln on the gathered [128,3,30].
Dense work that remains: softplus over all obj logits (67KB).

Key tricks:
- one manual ACT table load of set 6 (natural_log_exp_and_others) serves
  every Exp and Ln (and Abs): auto-placement otherwise ping-pongs
  exp/ln tables at 1.28us per load.
- single-reduce winner+minlab: minv_p = min_q [ ne*BIG + lab_q - 64*utri ].
  Same-cell later boxes contribute lab-64 < 0, so win = (minv >= 0) and
  minv == min-label exactly when p is a winner. One [128,128] compare and
  one reduce per scale instead of two of each.
- boxes/labels/consts packed into one [128,50] DMA; labels pre-cast to f32.
- output is the raw [128,18] per-box partial matrix; host masks nothing
  (win already multiplied on device) and just sums.
"""

import numpy as np

import concourse.bass as bass
import concourse.tile as tile
from concourse import bacc, mybir
from concourse.tile_rust import add_dep_helper

F32 = mybir.dt.float32
I32 = mybir.dt.int32
AF = mybir.ActivationFunctionType
OP = mybir.AluOpType
AX = mybir.AxisListType

B_TOT = 16
N_CORES = 8
B_SH = B_TOT // N_CORES
NBOX = 64
NP = B_SH * NBOX  # 128 partitions: (image, box)
C = 30
SCALES = [(80, 80), (40, 40), (20, 20)]
BIG = 1.0e9
REC_W = 5 + C  # obj, reg0..3, cls0..29
N_CELLS = sum(B_SH * h * w for h, w in SCALES)  # 16800
REC_BASE = [0, B_SH * 6400, B_SH * 6400 + B_SH * 1600]

CLS_W, REG_W, OBJ_W = 1.0, 5.0, 1.0
NPART = 18  # per scale s, cols 6s + [lse, clsval, sl1, obj, softplus, npos]

# pack layout: [kc (45) | boxes (4) | labels-as-f32 (1)] = 50 cols
KC_W = 15 + C
PK_W = KC_W + 5

# act_info.json set 6 = natural_log_exp_and_others: serves Exp, Ln, Abs
ACT_SET_EXP_LN = 6


def _kc_const():
    p = np.arange(128)
    bvec = (p >= NBOX).astype(np.float32)
    kc = np.zeros((128, KC_W), np.float32)
    for s, (h, w) in enumerate(SCALES):
        hw = h * w
        kc[:, 0 + s] = w          # x multiplier
        kc[:, 3 + s] = h          # y multiplier
        kc[:, 6 + s] = w - 1      # x clip
        kc[:, 9 + s] = h - 1      # y clip
        kc[:, 12 + s] = bvec * hw + REC_BASE[s]  # record-row offset
    kc[:, 15:] = np.arange(C, dtype=np.float32)[None, :]  # iota over classes
    return kc


def _big_const():
    ident = np.eye(128, dtype=np.float32)
    m64utri = -64.0 * np.triu(np.ones((128, 128), np.float32), 1)
    return np.concatenate([ident, m64utri], axis=1)  # [128, 256]


def emit(tc: tile.TileContext, outs, ins):
    """outs: partials AP [128,18]; ins: dict name -> AP (per-core shapes)."""
    nc = tc.nc

    big_h = nc.inline_tensor(_big_const(), name="cbig")

    pools = []

    def mkpool(**kw):
        p = tc.alloc_tile_pool(**kw)
        pools.append(p)
        return p

    pool = mkpool(name="sb", bufs=1)
    tps = mkpool(name="tps", bufs=1, space="PSUM")

    # ---- single activation-table load serving all Exp AND Ln ops ----
    nc.scalar.add_instruction(mybir.InstLoadActFuncSet(
        name=nc.scalar.bass.get_next_instruction_name(),
        act_func_set_id=ACT_SET_EXP_LN,
        engine=mybir.EngineType.Activation, ins=[], outs=[]))

    # ---- input loads ----
    pk = pool.tile([128, PK_W], F32, tag="pk")
    nc.sync.dma_start(out=pk[:], in_=ins["pack"])
    kct = pk[:, 0:KC_W]
    btile = pk[:, KC_W : KC_W + 4]
    labf = pk[:, KC_W + 4 : KC_W + 5]
    # gpsimd queue (idle until the gathers): transpose identity + -64*utri
    bigt = pool.tile([128, 256], F32, tag="bigt")
    nc.gpsimd.dma_start(out=bigt[:], in_=big_h.ap())
    m64utri = bigt[:, 128:256]
    # scalar queue: dense obj logits
    objt = []
    for s, (h, w) in enumerate(SCALES):
        n = B_SH * h * w
        p_obj = 128 if s < 2 else 32
        t = pool.tile([p_obj, n // p_obj], F32, tag=f"objt{s}")
        nc.scalar.dma_start(out=t[:], in_=ins[f"obj{s}"].rearrange("(p f) -> p f", p=p_obj))
        objt.append((p_obj, t))

    stack = pool.tile([128, NPART], F32, tag="stack")
    nc.vector.memset(stack[:], 0.0)
    stv = stack[:].rearrange("p (s j) -> p s j", j=6)

    # ---- box -> cell/key indices (vector) ----
    # floor(x) = round-to-nearest(x - 0.5): HW f32->i32 convert rounds.
    kxy = kct[:, 0:6].rearrange("p (c s) -> p c s", c=2)
    kxy_clip = kct[:, 6:12].rearrange("p (c s) -> p c s", c=2)
    gr = pool.tile([NP, 2, 3], F32, tag="gr")
    nc.vector.tensor_tensor(
        out=gr[:], in0=btile[:, 0:2, None].to_broadcast([NP, 2, 3]), in1=kxy, op=OP.mult
    )
    nc.vector.tensor_scalar(out=gr[:], in0=gr[:], scalar1=-0.5, scalar2=None, op0=OP.add)
    gi = pool.tile([NP, 2, 3], I32, tag="gi")
    nc.vector.tensor_copy(out=gi[:], in_=gr[:])
    gf = pool.tile([NP, 2, 3], F32, tag="gf")
    nc.vector.tensor_copy(out=gf[:], in_=gi[:])
    nc.vector.tensor_tensor(out=gf[:], in0=gf[:], in1=kxy_clip, op=OP.min)

    keyf = pool.tile([NP, 3], F32, tag="keyf")
    nc.vector.tensor_tensor(out=keyf[:], in0=gf[:, 1, :], in1=kct[:, 0:3], op=OP.mult)
    nc.vector.tensor_add(keyf[:], keyf[:], gf[:, 0, :])
    nc.vector.tensor_add(keyf[:], keyf[:], kct[:, 12:15])
    keyi = pool.tile([NP, 3], I32, tag="keyi")
    keyi_i = nc.vector.tensor_copy(out=keyi[:], in_=keyf[:])

    # ---- record gathers (gpsimd): indirect DMA honors ONE offset per
    # partition, so one [128, 35] gather per scale ----
    rows = pool.tile([NP, 3, REC_W], F32, tag="rows")
    for s in range(3):
        nc.gpsimd.indirect_dma_start(
            out=rows[:, s, :],
            out_offset=None,
            in_=ins["rec"],
            in_offset=bass.IndirectOffsetOnAxis(ap=keyi[:, s : s + 1], axis=0),
        )

    # ---- PE broadcast-transposes: labmat first, then per-scale keys ----
    labmat = tps.tile([128, 128], F32, tag="labmat")
    nc.tensor.transpose(out=labmat[:], in_=labf.to_broadcast([128, 128]), identity=bigt[:, 0:128])
    kmat = []
    for s in range(3):
        km = tps.tile([128, 128], F32, tag=f"kmat{s}")
        nc.tensor.transpose(
            out=km[:], in_=keyf[:, s : s + 1].to_broadcast([128, 128]), identity=bigt[:, 0:128]
        )
        kmat.append(km)

    # ---- dense obj softplus (scalar; exp then ln(1+y) with accum) ----
    for s, (p_obj, t) in enumerate(objt):
        obje = pool.tile([p_obj, t.shape[1]], F32, tag=f"obje{s}")
        nc.scalar.activation(out=obje[:], in_=t[:], func=AF.Exp)
        objl = pool.tile([p_obj, t.shape[1]], F32, tag=f"objl{s}")
        nc.scalar.activation(
            out=objl[:], in_=obje[:], func=AF.Ln, bias=1.0,
            accum_out=stack[:p_obj, 6 * s + 4 : 6 * s + 5],
        )

    # ---- same-cell masks (vector): single reduce gives winner AND minlab ----
    # lu[p,q] = lab_q - 64*utri[p,q]
    # minv_p  = min_q [ (key_q != key_p)*BIG + lu[p,q] ]
    #   winner (no later same-cell box): minv = min-label in [0, 30)
    #   loser: minv = lab_j - 64 in [-64, -35)  -> win = (minv >= 0)
    lu = pool.tile([128, 128], F32, tag="lu")
    lu_i = nc.vector.tensor_tensor(out=lu[:], in0=labmat[:], in1=m64utri, op=OP.add)
    # keep the scheduler from hoisting lu (waits on the labmat matmul) into
    # the middle of the box chain - it head-of-line blocks keyi otherwise
    add_dep_helper(lu_i.ins, keyi_i.ins, reason="order: box chain first")
    minv3 = pool.tile([NP, 3], F32, tag="minv3")
    for s in range(3):
        ne = pool.tile([128, 128], F32, tag=f"ne{s}")
        nc.vector.tensor_scalar(
            out=ne[:], in0=kmat[s][:], scalar1=keyf[:, s : s + 1], scalar2=BIG,
            op0=OP.not_equal, op1=OP.mult,
        )
        nc.vector.tensor_tensor(out=ne[:], in0=ne[:], in1=lu[:], op=OP.add)
        nc.vector.tensor_reduce(out=minv3[:, s : s + 1], in_=ne[:], axis=AX.X, op=OP.min)
    win3 = pool.tile([NP, 3], F32, tag="win3")
    nc.vector.tensor_scalar(out=win3[:], in0=minv3[:], scalar1=0.0, scalar2=None, op0=OP.is_ge)
    nc.vector.tensor_copy(out=stv[:, :, 5], in_=win3[:])

    # ---- CE: lse at cells (scalar exp+accum, ln) + logit at min-label ----
    se3 = pool.tile([NP, 3], F32, tag="se3")
    rexp = pool.tile([NP, 3, C], F32, tag="rexp")
    for s in range(3):
        nc.scalar.activation(
            out=rexp[:, s, :], in_=rows[:, s, 5:], func=AF.Exp,
            accum_out=se3[:, s : s + 1],
        )
    nc.scalar.activation(out=stv[:, :, 0], in_=se3[:], func=AF.Ln)

    # ---- smooth-L1 over gathered reg records (vector; |d| via max(d,-d)) ----
    d12 = pool.tile([NP, 3, 4], F32, tag="d12")
    nc.vector.tensor_tensor(
        out=d12[:], in0=rows[:, :, 1:5], in1=btile[:, None, :].to_broadcast([NP, 3, 4]), op=OP.subtract
    )
    dn12 = pool.tile([NP, 3, 4], F32, tag="dn12")
    nc.vector.tensor_tensor(
        out=dn12[:], in0=btile[:, None, :].to_broadcast([NP, 3, 4]), in1=rows[:, :, 1:5], op=OP.subtract
    )
    nc.vector.tensor_tensor(out=d12[:], in0=d12[:], in1=dn12[:], op=OP.max)
    q12 = pool.tile([NP, 3, 4], F32, tag="q12")
    nc.vector.tensor_scalar_min(q12[:], d12[:], 1.0)
    h12 = pool.tile([NP, 3, 4], F32, tag="h12")
    nc.vector.tensor_scalar(out=h12[:], in0=q12[:], scalar1=-0.5, scalar2=None, op0=OP.mult)
    nc.vector.tensor_add(h12[:], h12[:], d12[:])
    nc.vector.tensor_mul(h12[:], h12[:], q12[:])
    sl13 = pool.tile([NP, 3], F32, tag="sl13")
    nc.vector.tensor_reduce(out=sl13[:], in_=h12[:], axis=AX.X, op=OP.add)
    nc.vector.tensor_scalar(out=stv[:, :, 2], in0=sl13[:], scalar1=0.25, scalar2=10.0, op0=OP.mult, op1=OP.min)
    # obj logit at cell
    nc.vector.tensor_copy(out=stv[:, :, 3], in_=rows[:, :, 0])

    # ---- cls logit at min-label (0 for losers: minv < 0 never matches iota) ----
    sel3 = pool.tile([NP, 3, C], F32, tag="sel3")
    nc.vector.tensor_tensor(
        out=sel3[:], in0=kct[:, None, 15:].to_broadcast([NP, 3, C]),
        in1=minv3[:, :, None].to_broadcast([NP, 3, C]), op=OP.is_equal,
    )
    nc.vector.tensor_tensor(out=sel3[:], in0=sel3[:], in1=rows[:, :, 5:], op=OP.mult)
    nc.vector.tensor_reduce(out=stv[:, :, 1], in_=sel3[:], axis=AX.X, op=OP.add)

    # ---- ship the raw per-box partials transposed to [18,128] (18 DMA
    # descriptors instead of 128); host does the win-weighted sum ----
    finT = tps.tile([NPART, 128], F32, tag="finT")
    nc.tensor.transpose(out=finT[:], in_=stack[:], identity=bigt[:, 0:128])
    fin_sb = pool.tile([NPART, 128], F32, tag="fin_sb")
    nc.vector.tensor_copy(out=fin_sb[:], in_=finT[:])
    nc.sync.dma_start(out=outs, in_=fin_sb[:])

    for p in reversed(pools):
        p.release()


# ---------------------------------------------------------------------------
# host side
# ---------------------------------------------------------------------------

_CACHE = {}


def _build():
    if "nc" in _CACHE:
        return _CACHE["nc"]
    nc = bacc.Bacc(
        "TRN2",
        target_bir_lowering=False,
        debug=False,
        enable_asserts=False,
        num_devices=N_CORES,
    )
    ins = {}
    ins["rec"] = nc.dram_tensor("rec", (N_CELLS, REC_W), F32, kind="ExternalInput").ap()
    for s, (h, w) in enumerate(SCALES):
        ins[f"obj{s}"] = nc.dram_tensor(f"obj{s}", (B_SH * h * w,), F32, kind="ExternalInput").ap()
    ins["pack"] = nc.dram_tensor("pack", (128, PK_W), F32, kind="ExternalInput").ap()
    out = nc.dram_tensor("partials", (NPART, 128), F32, kind="ExternalOutput").ap()

    with tile.TileContext(nc) as tc:
        emit(tc, out, ins)
    nc.compile()
    _CACHE["nc"] = nc
    return nc


def make_records(inputs):
    """Full-batch per-cell records [B, HW_s, 35]: obj, reg0..3, cls0..29."""
    per_scale = []
    for s, (h, w) in enumerate(SCALES):
        hw = h * w
        rec = np.empty((B_TOT, hw, REC_W), np.float32)
        rec[:, :, 0] = np.asarray(inputs[f"obj_p{s}"]).reshape(B_TOT, hw)
        rec[:, :, 1:5] = np.asarray(inputs[f"reg_p{s}"]).reshape(B_TOT, 4, hw).transpose(0, 2, 1)
        rec[:, :, 5:] = np.asarray(inputs[f"cls_p{s}"]).reshape(B_TOT, C, hw).transpose(0, 2, 1)
        per_scale.append(rec)
    return per_scale


def combine_partials(parts):
    """parts: [n_cores, 18, 128] raw per-box partials -> final [4] losses.
    Device ships unmasked values; the win flag (row 6s+5) weights them here."""
    p = np.asarray(parts, np.float64).transpose(0, 2, 1).reshape(-1, 3, 6)
    win = p[:, :, 5:6]
    tot = np.concatenate([(p[:, :, 0:4] * win), p[:, :, 4:6]], axis=2).sum(axis=0)
    cls_sum = reg_sum = obj_sum = 0.0
    for s, (h, w) in enumerate(SCALES):
        lse, val, sl1, obj, sp, npos = tot[s]
        npos = max(npos, 1.0)
        cls_sum += (lse - val) / npos * CLS_W
        reg_sum += sl1 / npos * REG_W
        obj_sum += (sp - obj) / (B_TOT * h * w) * OBJ_W
    cls_sum /= len(SCALES)
    reg_sum /= len(SCALES)
    obj_sum /= len(SCALES)
    total = cls_sum + reg_sum + obj_sum
    return np.array([total, cls_sum, reg_sum, obj_sum], np.float32)


TRACE = False
LAST_RESULT = None

_KC = _kc_const()


def kernel(**inputs):
    global LAST_RESULT
    from concourse.bass_utils import run_bass_kernel_spmd

    nc = _build()
    per_scale = make_records(inputs)
    boxes = np.asarray(inputs["boxes"], np.float32)
    labels = np.asarray(inputs["labels"])
    in_maps = []
    for c in range(N_CORES):
        lo, hi = c * B_SH, (c + 1) * B_SH
        m = {}
        m["rec"] = np.concatenate(
            [ps[lo:hi].reshape(-1, REC_W) for ps in per_scale], axis=0
        )
        for s in range(3):
            m[f"obj{s}"] = np.ascontiguousarray(
                np.asarray(inputs[f"obj_p{s}"][lo:hi]).reshape(-1)
            )
        pack = np.empty((128, PK_W), np.float32)
        pack[:, :KC_W] = _KC
        pack[:, KC_W : KC_W + 4] = boxes[lo:hi].reshape(NP, 4)
        pack[:, KC_W + 4] = labels[lo:hi].reshape(NP).astype(np.float32)
        m["pack"] = pack
        in_maps.append(m)
    res = run_bass_kernel_spmd(
        nc, in_maps, core_ids=list(range(N_CORES)), trace=TRACE
    )
    LAST_RESULT = res
    parts = np.stack([np.asarray(r["partials"]) for r in res.results])
    return combine_partials(parts)
